# revision 6
# baseline (speedup 1.0000x reference)
"""Trainium2 Bass kernel for the ChebConv GNN (nn_Cheb_13116830122345).

Sharding: nodes (and their incident edges, by dst) partitioned across 8 cores.
Per layer: X1 = -prop(X0) computed via indirect-DMA row gathers from a
replicated y-table (y = dinv*X0, AllGather'ed each layer) + on-chip segment
reduction through PE matmuls against on-device-built (-dinv[dst]) one-hot
selection matrices.  ChebConv linear runs feature-major on PE; BatchNorm batch
stats are per-partition reductions + a tiny AllReduce; SumPooling is a PE
matmul against a graph one-hot, partial per core, summed on host.  The final
(tiny) prediction head / log_softmax / mean over [512,10]/[512,128] is
finished on host.
"""
import sys
import time

sys.path.insert(0, "/opt/trn_rl_repo")

import numpy as np

N = 100000
E = 1600000
G = 512
H = 128
L = 5
EPS = 1e-5
NCORES = 8
OWN = N // NCORES           # 12500 real nodes per core
PPC = 12544                 # padded nodes per core (98 * 128)
NTILE = PPC // 128          # 98
NP = PPC * NCORES           # padded total rows in the y table
PAD = PPC - OWN             # 44 pad rows per core

LAST_REAL = OWN - (NTILE - 1) * 128  # real nodes in the last tile (84)

timings = {}


def _host_prepack(feat, src, dst, node2graph, dinv):
    """Per-core edge chunking + constant arrays. Returns list of in_maps plus meta."""
    src = np.asarray(src).astype(np.int64)
    dst = np.asarray(dst).astype(np.int64)
    n2g = np.asarray(node2graph).astype(np.int64)

    core = dst // OWN                      # owner core of each edge (by dst)
    ln = dst - core * OWN                  # local node id in [0, OWN)
    tile = ln // 128
    rel = ln - tile * 128

    # global padded row id of a node
    def gpad(ids):
        return ids + PAD * (ids // OWN)

    src_p = gpad(src)

    # per-tile chunk count: max over cores (SPMD needs tile-wise uniformity only)
    counts = np.zeros((NCORES, NTILE), np.int64)
    np.add.at(counts, (core, tile), 1)
    ncht = np.ceil(counts.max(axis=0) / 128).astype(np.int64)   # [NTILE]
    cum = np.zeros(NTILE + 1, np.int64)
    cum[1:] = np.cumsum(ncht)
    slots = int(cum[-1])

    order = np.lexsort((tile, core))
    core_s, tile_s, rel_s, srcp_s = core[order], tile[order], rel[order], src_p[order]
    ndinv_s = -dinv[dst[order]]

    in_maps = []
    for c in range(NCORES):
        srcidx = np.full((128, slots), c * PPC + OWN, np.int32)  # pad row (always 0)
        ndv = np.zeros((128, slots), np.float32)
        drel = np.zeros((128, slots), np.float32)
        sel = core_s == c
        t_c, r_c, s_c, nd_c = tile_s[sel], rel_s[sel], srcp_s[sel], ndinv_s[sel]
        # position within tile
        start = np.searchsorted(t_c, np.arange(NTILE))
        for t in range(NTILE):
            a, b = start[t], (start[t + 1] if t + 1 < NTILE else len(t_c))
            cnt = b - a
            pos = np.arange(cnt)
            col = int(cum[t]) + pos // 128
            row = pos % 128
            srcidx[row, col] = s_c[a:b]
            ndv[row, col] = nd_c[a:b]
            drel[row, col] = r_c[a:b]
        # graph-rel column per tile: [128, NTILE]
        lids = np.arange(PPC)
        real = lids < OWN
        g0 = n2g[c * OWN]
        grel_full = np.full(PPC, -1.0, np.float32)
        grel_full[real] = (n2g[c * OWN:(c + 1) * OWN] - g0).astype(np.float32)
        gspan = int(grel_full.max()) + 1
        assert gspan <= 128, f"core {c} spans {gspan} graphs"
        grel = grel_full.reshape(NTILE, 128).T.copy()
        dcol_full = np.zeros(PPC, np.float32)
        dcol_full[real] = dinv[c * OWN:(c + 1) * OWN]
        dcol = dcol_full.reshape(NTILE, 128).T.copy()
        y0 = np.zeros((PPC, H), np.float32)
        y0[:OWN] = feat[c * OWN:(c + 1) * OWN] * dinv[c * OWN:(c + 1) * OWN, None]
        featT = np.zeros((128, PPC), np.float32)
        featT[:, :OWN] = feat[c * OWN:(c + 1) * OWN].T
        in_maps.append(dict(srcidx=srcidx, ndinv=ndv, dstrel=drel, grel=grel,
                            dinv_col=dcol, y0=y0, featT=featT, g0=int(g0)))
    return in_maps, ncht, cum, slots


def _build_nc(ncht, cum, slots, reps=1):
    import os
    nocoll = os.environ.get("NOCOLL", "0") == "1"
    from concourse import bass, bacc, mybir
    import concourse.tile as tile
    from concourse.masks import make_identity

    DT = mybir.dt.float32

    nc = bacc.Bacc("TRN2")
    # ---- I/O ----
    t_y0 = nc.dram_tensor("y0", [PPC, H], DT, kind="ExternalInput")
    t_featT = nc.dram_tensor("featT", [128, PPC], DT, kind="ExternalInput")
    t_srcidx = nc.dram_tensor("srcidx", [128, slots], mybir.dt.int32, kind="ExternalInput")
    t_ndinv = nc.dram_tensor("ndinv", [128, slots], DT, kind="ExternalInput")
    t_dstrel = nc.dram_tensor("dstrel", [128, slots], DT, kind="ExternalInput")
    t_grel = nc.dram_tensor("grel", [128, NTILE], DT, kind="ExternalInput")
    t_dinv_col = nc.dram_tensor("dinv_col", [128, NTILE], DT, kind="ExternalInput")
    t_cheb = nc.dram_tensor("cheb", [128, L * 256], DT, kind="ExternalInput")
    t_cols = nc.dram_tensor("cols", [128, 3 * L], DT, kind="ExternalInput")  # b, gamma, beta
    t_iota = nc.dram_tensor("iota", [128, 128], DT, kind="ExternalInput")
    t_pool_out = nc.dram_tensor("pooled", [128, L * 128], DT, kind="ExternalOutput")

    Relu = mybir.ActivationFunctionType.Relu
    Square = mybir.ActivationFunctionType.Square
    Sqrt = mybir.ActivationFunctionType.Sqrt
    ALU = mybir.AluOpType
    AX = mybir.AxisListType.X

    with tile.TileContext(nc) as tc:
        with (
            tc.tile_pool(name="persist", bufs=1) as pp,
            tc.tile_pool(name="work", bufs=3) as wp,
            tc.tile_pool(name="deep", bufs=6) as dpp,
            tc.tile_pool(name="psum", bufs=2, space="PSUM") as psp,
            tc.tile_pool(name="psum1", bufs=1, space="PSUM") as ps1,
            tc.tile_pool(name="dram", bufs=1, space="DRAM") as dp,
        ):
            y_loc = dp.tile([PPC, H], DT, tag="y_loc")
            y_full = dp.tile([NP, H], DT, tag="y_full")
            st_in = dp.tile([128, 2], DT, tag="st_in")
            st_out = dp.tile([128, 2], DT, tag="st_out")

            XT = pp.tile([128, PPC], DT, tag="XT")       # X0^T (feature-major)
            hT = pp.tile([128, PPC], DT, tag="hT")       # raw h^T
            sidx = pp.tile([128, slots], mybir.dt.int32, tag="sidx")
            sndv = pp.tile([128, slots], DT, tag="sndv")
            sdrel = pp.tile([128, slots], DT, tag="sdrel")
            sgrel = pp.tile([128, NTILE], DT, tag="sgrel")
            sdinv = pp.tile([128, NTILE], DT, tag="sdinv")
            scheb = pp.tile([128, L * 256], DT, tag="scheb")
            scols = pp.tile([128, 3 * L], DT, tag="scols")
            siota = pp.tile([128, 128], DT, tag="siota")
            ident = pp.tile([128, 128], DT, tag="ident")
            s1col = pp.tile([128, NTILE], DT, tag="s1col")
            s2col = pp.tile([128, NTILE], DT, tag="s2col")
            sstat = pp.tile([128, 2], DT, tag="sstat")
            s_a = pp.tile([128, 1], DT, tag="s_a")
            s_shift = pp.tile([128, 1], DT, tag="s_shift")
            seps = pp.tile([128, 1], DT, tag="seps")

            nc.gpsimd.memset(seps[:], EPS)
            make_identity(nc, ident[:])
            nc.sync.dma_start(siota[:], t_iota[:, :])
            nc.sync.dma_start(sidx[:], t_srcidx[:, :])
            nc.sync.dma_start(sndv[:], t_ndinv[:, :])
            nc.sync.dma_start(sdrel[:], t_dstrel[:, :])
            nc.sync.dma_start(sgrel[:], t_grel[:, :])
            nc.sync.dma_start(sdinv[:], t_dinv_col[:, :])
            nc.sync.dma_start(scheb[:], t_cheb[:, :])
            nc.sync.dma_start(scols[:], t_cols[:, :])
            nc.sync.dma_start(XT[:], t_featT[:, :])
            nc.sync.dma_start(y_loc[:], t_y0[:, :])

            def allgather_y():
                if nocoll:
                    nc.sync.dma_start(y_full[0:PPC, :], y_loc[:])
                    return
                nc.gpsimd.collective_compute(
                    "AllGather", ALU.bypass,
                    replica_groups=[list(range(NCORES))],
                    ins=[y_loc[:].opt()], outs=[y_full[:].opt()],
                )

            allgather_y()

            for _rep in range(reps):
              for l in range(L):
                # ---------- P2: h = Wt.T @ X0T + Wb.T @ X1T ----------
                for t in range(NTILE):
                    nch_t = int(ncht[t])
                    x1p = psp.tile([128, 128], DT, tag="x1p", space="PSUM")
                    for k in range(nch_t):
                        col = int(cum[t]) + k
                        g = dpp.tile([128, H], DT, tag="gbuf")
                        nc.gpsimd.indirect_dma_start(
                            out=g[:, :], out_offset=None, in_=y_full[:, :],
                            in_offset=bass.IndirectOffsetOnAxis(
                                ap=sidx[:, col:col + 1], axis=0),
                        )
                        m = dpp.tile([128, 128], DT, tag="mbuf")
                        nc.vector.tensor_scalar(
                            out=m[:], in0=siota[:],
                            scalar1=sdrel[:, col:col + 1],
                            scalar2=sndv[:, col:col + 1],
                            op0=ALU.is_equal, op1=ALU.mult,
                        )
                        nc.tensor.matmul(x1p[:], lhsT=g[:], rhs=m[:],
                                         start=(k == 0), stop=(k == nch_t - 1))
                    x1s = wp.tile([128, 128], DT, tag="x1s")
                    nc.vector.tensor_copy(out=x1s[:], in_=x1p[:])
                    hp = psp.tile([128, 128], DT, tag="hp", space="PSUM")
                    nc.tensor.matmul(hp[:], lhsT=scheb[:, l * 256:l * 256 + 128],
                                     rhs=XT[:, t * 128:(t + 1) * 128],
                                     start=True, stop=False)
                    nc.tensor.matmul(hp[:], lhsT=scheb[:, l * 256 + 128:l * 256 + 256],
                                     rhs=x1s[:], start=False, stop=True)
                    nc.vector.tensor_copy(out=hT[:, t * 128:(t + 1) * 128], in_=hp[:])
                    cnt = LAST_REAL if t == NTILE - 1 else 128
                    nc.vector.reduce_sum(out=s1col[:, t:t + 1],
                                         in_=hT[:, t * 128:t * 128 + cnt], axis=AX)
                    sq = wp.tile([128, 128], DT, tag="sqbuf")
                    nc.scalar.activation(out=sq[:, :cnt],
                                         in_=hT[:, t * 128:t * 128 + cnt],
                                         func=Square, accum_out=s2col[:, t:t + 1])
                # stats: local sums -> AllReduce
                nc.vector.reduce_sum(out=sstat[:, 0:1], in_=s1col[:, :], axis=AX)
                nc.vector.reduce_sum(out=sstat[:, 1:2], in_=s2col[:, :], axis=AX)
                nc.sync.dma_start(st_in[:], sstat[:])
                if nocoll:
                    nc.sync.dma_start(st_out[:], st_in[:])
                else:
                    nc.gpsimd.collective_compute(
                        "AllReduce", ALU.add,
                        replica_groups=[list(range(NCORES))],
                        ins=[st_in[:].opt()], outs=[st_out[:].opt()],
                    )
                nc.sync.dma_start(sstat[:], st_out[:])
                # BN affine:  a = gamma / sqrt(var+eps);  shift = beta - (mu_raw + b) * a
                mu = wp.tile([128, 1], DT, tag="bn0")
                msq = wp.tile([128, 1], DT, tag="bn1")
                var = wp.tile([128, 1], DT, tag="bn2")
                tmp = wp.tile([128, 1], DT, tag="bn3")
                nc.vector.tensor_scalar(out=mu[:], in0=sstat[:, 0:1],
                                        scalar1=1.0 / N, scalar2=None, op0=ALU.mult)
                nc.vector.tensor_scalar(out=msq[:], in0=sstat[:, 1:2],
                                        scalar1=1.0 / N, scalar2=None, op0=ALU.mult)
                nc.vector.tensor_tensor(out=var[:], in0=mu[:], in1=mu[:], op=ALU.mult)
                nc.vector.tensor_tensor(out=var[:], in0=msq[:], in1=var[:], op=ALU.subtract)
                nc.scalar.activation(out=var[:], in_=var[:], func=Sqrt, bias=seps[:, 0:1])
                nc.vector.reciprocal(out=tmp[:], in_=var[:])
                nc.vector.tensor_tensor(out=s_a[:], in0=tmp[:],
                                        in1=scols[:, L + l:L + l + 1], op=ALU.mult)
                nc.vector.tensor_tensor(out=mu[:], in0=mu[:],
                                        in1=scols[:, l:l + 1], op=ALU.add)
                nc.vector.tensor_tensor(out=tmp[:], in0=mu[:], in1=s_a[:], op=ALU.mult)
                nc.vector.tensor_tensor(out=s_shift[:], in0=scols[:, 2 * L + l:2 * L + l + 1],
                                        in1=tmp[:], op=ALU.subtract)
                # ---------- P1 of layer l+1: normalize, pool rep l+1, write y ----------
                poolp = ps1.tile([128, 128], DT, tag="poolp", space="PSUM")
                for t in range(NTILE):
                    nc.scalar.activation(out=XT[:, t * 128:(t + 1) * 128],
                                         in_=hT[:, t * 128:(t + 1) * 128],
                                         func=Relu, bias=s_shift[:, 0:1],
                                         scale=s_a[:, 0:1])
                    trp = psp.tile([128, 128], DT, tag="trp", space="PSUM")
                    nc.tensor.transpose(out=trp[:], in_=XT[:, t * 128:(t + 1) * 128],
                                        identity=ident[:])
                    x0nm = wp.tile([128, 128], DT, tag="x0nm")
                    nc.vector.tensor_copy(out=x0nm[:], in_=trp[:])
                    pm = wp.tile([128, 128], DT, tag="pmbuf")
                    nc.vector.tensor_scalar(out=pm[:], in0=siota[:],
                                            scalar1=sgrel[:, t:t + 1], scalar2=None,
                                            op0=ALU.is_equal)
                    nc.tensor.matmul(poolp[:], lhsT=x0nm[:], rhs=pm[:],
                                     start=(t == 0), stop=(t == NTILE - 1))
                    if l < L - 1:
                        yb = wp.tile([128, 128], DT, tag="ybuf")
                        nc.vector.tensor_scalar(out=yb[:], in0=x0nm[:],
                                                scalar1=sdinv[:, t:t + 1],
                                                scalar2=None, op0=ALU.mult)
                        nc.sync.dma_start(y_loc[t * 128:(t + 1) * 128, :], yb[:])
                pout = wp.tile([128, 128], DT, tag="pout")
                nc.vector.tensor_copy(out=pout[:], in_=poolp[:])
                nc.sync.dma_start(t_pool_out[:, l * 128:(l + 1) * 128], pout[:])
                if l < L - 1:
                    allgather_y()

    nc.compile()
    return nc


def kernel(feat, cheb_W, cheb_b, bn_gamma, bn_beta, pred_W, pred_b,
           src, dst, node2graph):
    from concourse.bass_utils import run_bass_kernel_spmd

    feat = np.asarray(feat, np.float32)
    cheb_W = np.asarray(cheb_W, np.float32)
    cheb_b = np.asarray(cheb_b, np.float32)
    bn_gamma = np.asarray(bn_gamma, np.float32)
    bn_beta = np.asarray(bn_beta, np.float32)
    pred_W = np.asarray(pred_W, np.float32)
    pred_b = np.asarray(pred_b, np.float32)
    srci = np.asarray(src).astype(np.int64)
    dsti = np.asarray(dst).astype(np.int64)
    n2g = np.asarray(node2graph).astype(np.int64)

    t0 = time.time()
    deg = np.bincount(dsti, minlength=N).astype(np.float32)
    dinv = np.clip(deg, 1.0, None) ** -0.5
    per_core, ncht, cum, slots = _host_prepack(feat, srci, dsti, n2g, dinv)
    timings["slots"] = slots
    timings["prepack_s"] = time.time() - t0

    # shared constant inputs
    iota = np.tile(np.arange(128, dtype=np.float32)[None, :], (128, 1))
    cheb = np.concatenate(
        [np.concatenate([cheb_W[l][:128], cheb_W[l][128:]], axis=1) for l in range(L)],
        axis=1)  # [128, L*256]
    cols = np.concatenate(
        [cheb_b.T, bn_gamma.T, bn_beta.T], axis=1).astype(np.float32)  # [128, 3L]

    t0 = time.time()
    import os
    reps = int(os.environ.get("KREPS", "1"))
    nc = _build_nc(ncht, cum, slots, reps=reps)
    timings["build_s"] = time.time() - t0

    in_maps = []
    for c in range(NCORES):
        m = per_core[c]
        in_maps.append({
            "y0": m["y0"], "featT": m["featT"], "srcidx": m["srcidx"],
            "ndinv": m["ndinv"], "dstrel": m["dstrel"], "grel": m["grel"],
            "dinv_col": m["dinv_col"], "cheb": cheb, "cols": cols, "iota": iota,
        })

    t0 = time.time()
    res = run_bass_kernel_spmd(nc, in_maps, list(range(NCORES)))
    timings["first_run_s"] = time.time() - t0

    # ---- host finish ----
    pooled = np.zeros((L + 1, G, H), np.float32)
    np.add.at(pooled[0], n2g, feat)           # rep 0 pooled from raw input
    for c in range(NCORES):
        g0 = per_core[c]["g0"]
        pT = res.results[c]["pooled"]          # [128, L*128]
        for l in range(L):
            blk = pT[:, l * 128:(l + 1) * 128]  # [feat, graph-rel]
            ghi = min(128, G - g0)
            pooled[l + 1, g0:g0 + ghi] += blk[:, :ghi].T
    score = pred_b.sum(axis=0)[None, :] + np.zeros((G, 10), np.float32)
    for i in range(L + 1):
        score = score + pooled[i] @ pred_W[i]
    mx = score.max(axis=1, keepdims=True)
    ls = score - mx - np.log(np.exp(score - mx).sum(axis=1, keepdims=True))
    mean_pooled = pooled[1:].mean(axis=0)
    return ls.astype(np.float32), mean_pooled.astype(np.float32)
